# revision 1
# baseline (speedup 1.0000x reference)
"""MoE layer (straight-through, gate-token routing) on 8 trn2 NeuronCores.

Strategy:
  Launch 1 (gating, data-parallel): each core takes 512 tokens (x^T shard)
    and computes logits -> softmax -> argmax mask on device. Outputs per
    core: per-token expert ids, per-expert prob column-sums and counts
    (partition-reduced on the tensor engine via a ones-vector matmul).
  Host: shards tokens by expert id ("all-to-all" in host numpy), 2 cores
    per expert, fixed capacity C (padded with zero columns).
  Launch 2 (experts, expert-parallel, F-split): core 2e+h holds half of
    expert e's FFN (F/2 = 1536 columns of W1, matching rows of W2) and
    processes all of expert e's tokens:
        outT_part = W2h^T @ gelu(W1h^T @ xT + b1h)   (+ b2 on h==0 core)
    Matmuls run in fp32r (full-rate fp32 mode; inputs rounded on-chip).
    The two partial outputs of a pair are summed on the host (unshard of
    the F-split) and scattered back to token positions.
  balance_loss is computed on device (launch 2) from the globally summed
  gating partials, so every output value is device-computed.
"""

import sys

if "/opt/trn_rl_repo" not in sys.path:
    sys.path.insert(0, "/opt/trn_rl_repo")

import numpy as np

import concourse.bass as bass
import concourse.mybir as mybir
import concourse.tile as tile
from concourse import bacc
from concourse.bass_utils import run_bass_kernel_spmd

F32 = mybir.dt.float32
F32R = mybir.dt.float32r
AF = mybir.ActivationFunctionType
ALU = mybir.AluOpType
AX = mybir.AxisListType

B, S, D = 8, 512, 768
E, F = 4, 3072
N = B * S                 # 4096 tokens
TPC = N // 8              # 512 tokens per core in the gating launch
DC = D // 128             # 6 contraction chunks of 128
FH = F // 2               # 1536 F-columns per core (F-split across the pair)
FT = FH // 128            # 12 f-tiles per core
C_DEFAULT = 1152          # per-expert token capacity (mean load is 1024)

_CACHE = {}


def _chunks(c):
    """Split capacity C into matmul moving-dim chunks, each in [256, 512]
    (fp32r runs full-rate only for free dim >= 256)."""
    out = []
    rem = c
    while rem >= 1024:
        out.append(512)
        rem -= 512
    if rem == 512:
        out.append(512)
    elif rem > 512:
        h1 = ((rem // 2 + 127) // 128) * 128
        out.append(h1)
        out.append(rem - h1)
    elif rem > 0:
        out.append(rem)
    spans, t0 = [], 0
    for cn in out:
        spans.append((t0, cn))
        t0 += cn
    assert t0 == c
    return spans


def _build_gate():
    nc = bacc.Bacc("TRN2", target_bir_lowering=False, debug=False, num_devices=8)
    xT = nc.dram_tensor("xT", [D, TPC], F32, kind="ExternalInput")
    gwT = nc.dram_tensor("gwT", [D, E], F32, kind="ExternalInput")
    iota = nc.dram_tensor("iota", [128, E], F32, kind="ExternalInput")
    ids = nc.dram_tensor("ids", [128, TPC // 128], F32, kind="ExternalOutput")
    pc = nc.dram_tensor("pc", [2 * E, 1], F32, kind="ExternalOutput")

    xr = xT.ap().rearrange("(c p) t -> c p t", p=128)
    gr = gwT.ap().rearrange("(c p) e -> p c e", p=128)
    NT = TPC // 128

    with tile.TileContext(nc) as tc:
        with (
            tc.tile_pool(name="sb", bufs=2) as sb,
            tc.tile_pool(name="big", bufs=NT) as big,
            tc.tile_pool(name="ps", bufs=2, space="PSUM") as ps,
            tc.tile_pool(name="psc", bufs=1, space="PSUM") as psc,
        ):
            xs = []
            for dcI in range(DC):
                t = sb.tile([128, TPC], F32, tag=f"x{dcI}")
                nc.sync.dma_start(t[:], xr[dcI])
                xs.append(t)
            gw = sb.tile([128, DC, E], F32, tag="gw")
            nc.sync.dma_start(gw[:], gr[:])
            io = sb.tile([128, E], F32, tag="io")
            nc.sync.dma_start(io[:], iota[:])
            ones = sb.tile([128, 1], F32, tag="ones")
            nc.vector.memset(ones[:], 1.0)

            ids_sb = sb.tile([128, NT], F32, tag="ids")
            ppc = psc.tile([2 * E, 1], F32, tag="ppc")

            for tt in range(NT):
                pl = ps.tile([128, E], F32, tag="pl")
                for dcI in range(DC):
                    nc.tensor.matmul(
                        pl[:],
                        xs[dcI][:, tt * 128:(tt + 1) * 128],
                        gw[:, dcI, :],
                        start=(dcI == 0),
                        stop=(dcI == DC - 1),
                    )
                m = big.tile([128, 1], F32, tag="m")
                nc.vector.tensor_reduce(m[:], pl[:], AX.X, ALU.max)
                negm = big.tile([128, 1], F32, tag="negm")
                nc.scalar.mul(negm[:], m[:], -1.0)
                pm = big.tile([128, 2 * E], F32, tag="pm")
                s = big.tile([128, 1], F32, tag="s")
                nc.scalar.activation(
                    pm[:, 0:E], pl[:], AF.Exp, bias=negm[:], scale=1.0,
                    accum_out=s[:],
                )
                nc.vector.tensor_scalar(pm[:, E:2 * E], pl[:], m[:], None, ALU.is_ge)
                r = big.tile([128, 1], F32, tag="r")
                nc.vector.reciprocal(r[:], s[:])
                nc.vector.tensor_scalar_mul(pm[:, 0:E], pm[:, 0:E], r[:])
                tmp = big.tile([128, E], F32, tag="tmp")
                nc.vector.tensor_tensor(tmp[:], pm[:, E:2 * E], io[:], ALU.mult)
                nc.vector.tensor_reduce(ids_sb[:, tt:tt + 1], tmp[:], AX.X, ALU.add)
                nc.tensor.matmul(
                    ppc[:], pm[:], ones[:],
                    start=(tt == 0), stop=(tt == NT - 1),
                )

            pc_sb = sb.tile([2 * E, 1], F32, tag="pc")
            nc.vector.tensor_copy(pc_sb[:], ppc[:])
            nc.sync.dma_start(pc[:], pc_sb[:])
            nc.sync.dma_start(ids[:], ids_sb[:])
    nc.compile()
    return nc


def _build_expert(cap):
    spans = _chunks(cap)
    nc = bacc.Bacc("TRN2", target_bir_lowering=False, debug=False, num_devices=8)
    xT = nc.dram_tensor("xT", [D, cap], F32, kind="ExternalInput")
    w1 = nc.dram_tensor("w1", [D, FH], F32, kind="ExternalInput")
    b1 = nc.dram_tensor("b1", [128, FT], F32, kind="ExternalInput")
    w2 = nc.dram_tensor("w2", [FH, D], F32, kind="ExternalInput")
    b2 = nc.dram_tensor("b2", [128, DC], F32, kind="ExternalInput")
    pcin = nc.dram_tensor("pcin", [1, 2 * E], F32, kind="ExternalInput")
    outT = nc.dram_tensor("outT", [D, cap], F32, kind="ExternalOutput")
    bal = nc.dram_tensor("bal", [1, 1], F32, kind="ExternalOutput")

    xr = xT.ap().rearrange("(c p) t -> c p t", p=128)
    w1r_d = w1.ap().rearrange("(c p) f -> c p f", p=128)
    w2r_d = w2.ap().rearrange("(c p) d -> c p d", p=128)
    outr = outT.ap().rearrange("(c p) t -> c p t", p=128)

    with tile.TileContext(nc) as tc:
        with (
            tc.tile_pool(name="stage", bufs=2) as stage,
            tc.tile_pool(name="res", bufs=1) as res,
            tc.tile_pool(name="hp", bufs=1) as hp,
            tc.tile_pool(name="op", bufs=3) as op,
            tc.tile_pool(name="psA", bufs=3, space="PSUM") as psA,
            tc.tile_pool(name="psB", bufs=3, space="PSUM") as psB,
        ):
            b1_sb = res.tile([128, FT], F32, tag="b1")
            nc.sync.dma_start(b1_sb[:], b1.ap())
            b2_sb = res.tile([128, DC], F32, tag="b2")
            nc.sync.dma_start(b2_sb[:], b2.ap())

            # balance loss from global gating partials
            pc_sb = res.tile([1, 2 * E], F32, tag="pc")
            nc.sync.dma_start(pc_sb[:], pcin.ap())
            tmp4 = res.tile([1, E], F32, tag="tmp4")
            nc.vector.tensor_tensor(tmp4[:], pc_sb[0:1, 0:E], pc_sb[0:1, E:2 * E], ALU.mult)
            bsum = res.tile([1, 1], F32, tag="bsum")
            nc.vector.tensor_reduce(bsum[:], tmp4[:], AX.X, ALU.add)
            bal_sb = res.tile([1, 1], F32, tag="bal")
            nc.scalar.mul(bal_sb[:], bsum[:], float(E) / float(N) / float(N))
            nc.sync.dma_start(bal.ap(), bal_sb[:])

            # stream in + round to fp32r: x (DVE), W1 (DVE), W2 (ACT)
            xrr, w1t, w2t = [], [], []
            for dcI in range(DC):
                st = stage.tile([128, cap], F32, tag="xs")
                nc.sync.dma_start(st[:], xr[dcI])
                rt = res.tile([128, cap], F32R, tag=f"xr{dcI}")
                nc.vector.tensor_copy(rt[:], st[:])
                xrr.append(rt)
            for dcI in range(DC):
                st = stage.tile([128, FH], F32, tag="w1s")
                nc.sync.dma_start(st[:], w1r_d[dcI])
                rt = res.tile([128, FH], F32R, tag=f"w1r{dcI}")
                nc.vector.tensor_copy(rt[:], st[:])
                w1t.append(rt)
            for fc in range(FT):
                st = stage.tile([128, D], F32, tag="w2s")
                nc.sync.dma_start(st[:], w2r_d[fc])
                rt = res.tile([128, D], F32R, tag=f"w2r{fc}")
                nc.scalar.copy(rt[:], st[:])
                w2t.append(rt)

            # layer 1: h = gelu(W1h^T @ xT + b1h), written as fp32r by ACT
            hs = []
            for ft in range(FT):
                h = hp.tile([128, cap], F32R, tag=f"h{ft}")
                hs.append(h)
            for ft in range(FT):
                for (t0, tn) in spans:
                    ph = psA.tile([128, tn], F32, tag="ph")
                    for dcI in range(DC):
                        nc.tensor.matmul(
                            ph[:],
                            w1t[dcI][:, ft * 128:(ft + 1) * 128],
                            xrr[dcI][:, t0:t0 + tn],
                            start=(dcI == 0),
                            stop=(dcI == DC - 1),
                        )
                    nc.scalar.activation(
                        hs[ft][:, t0:t0 + tn], ph[:], AF.Gelu,
                        bias=b1_sb[:, ft:ft + 1], scale=1.0,
                    )

            # layer 2: outT = W2h^T @ h + b2 (d-major, per-partition bias)
            for (t0, tn) in spans:
                for db in range(DC):
                    po = psB.tile([128, tn], F32, tag="po")
                    for fc in range(FT):
                        nc.tensor.matmul(
                            po[:],
                            w2t[fc][:, db * 128:(db + 1) * 128],
                            hs[fc][:, t0:t0 + tn],
                            start=(fc == 0),
                            stop=(fc == FT - 1),
                        )
                    ot = op.tile([128, tn], F32, tag="ot")
                    nc.scalar.activation(
                        ot[:], po[:], AF.Identity, bias=b2_sb[:, db:db + 1], scale=1.0,
                    )
                    nc.sync.dma_start(outr[db][:, t0:t0 + tn], ot[:])
    nc.compile()
    return nc


def _get_gate():
    if "gate" not in _CACHE:
        _CACHE["gate"] = _build_gate()
    return _CACHE["gate"]


def _get_expert(cap):
    key = ("exp", cap)
    if key not in _CACHE:
        _CACHE[key] = _build_expert(cap)
    return _CACHE[key]


def _run(inputs, trace=False):
    x = np.ascontiguousarray(np.asarray(inputs["x"], dtype=np.float32))
    gate_w = np.asarray(inputs["gate_w"], dtype=np.float32)
    W1 = np.asarray(inputs["W1"], dtype=np.float32)
    b1 = np.asarray(inputs["b1"], dtype=np.float32)
    W2 = np.asarray(inputs["W2"], dtype=np.float32)
    b2 = np.asarray(inputs["b2"], dtype=np.float32)

    xf = x.reshape(N, D)
    xT = np.ascontiguousarray(xf.T)                      # [768, 4096]
    gwT = np.ascontiguousarray(gate_w.T)                 # [768, 4]
    iota = np.broadcast_to(
        np.arange(E, dtype=np.float32), (128, E)
    ).copy()

    # ---- launch 1: gating ----
    nc_g = _get_gate()
    in_maps = []
    for c in range(8):
        in_maps.append({
            "xT": np.ascontiguousarray(xT[:, c * TPC:(c + 1) * TPC]),
            "gwT": gwT,
            "iota": iota,
        })
    res_g = run_bass_kernel_spmd(nc_g, in_maps, core_ids=list(range(8)), trace=trace)
    t_gate = res_g.exec_time_ns

    gate = np.concatenate(
        [res_g.results[c]["ids"].T.reshape(-1) for c in range(8)]
    )
    gate = np.rint(gate).astype(np.int64)
    probsum = np.sum([res_g.results[c]["pc"][0:E, 0] for c in range(8)], axis=0)
    counts = np.sum([res_g.results[c]["pc"][E:2 * E, 0] for c in range(8)], axis=0)
    gate_load = np.rint(counts).astype(np.int32)

    # ---- host all-to-all by gate id ----
    idx = [np.flatnonzero(gate == e) for e in range(E)]
    max_load = max(len(i) for i in idx)
    cap = C_DEFAULT
    if max_load > cap:
        cap = ((max_load + 255) // 256) * 256
        if cap % 512 == 128:
            cap += 128
    nc_e = _get_expert(cap)

    pcin = np.concatenate([probsum, counts]).astype(np.float32)[None, :]
    zeros_b2 = np.zeros_like(b2[0])
    in_maps = []
    xsel = []
    for e in range(E):
        xs = np.zeros((D, cap), np.float32)
        xs[:, :len(idx[e])] = xT[:, idx[e]]
        xsel.append(xs)
    for core in range(8):
        e, h = core // 2, core % 2
        w1h = np.ascontiguousarray(W1[e][:, h * FH:(h + 1) * FH])
        b1h = np.ascontiguousarray(b1[e][h * FH:(h + 1) * FH].reshape(FT, 128).T)
        w2h = np.ascontiguousarray(W2[e][h * FH:(h + 1) * FH, :])
        b2h = b2[e] if h == 0 else zeros_b2
        b2h = np.ascontiguousarray(b2h.reshape(DC, 128).T)
        in_maps.append({
            "xT": xsel[e], "w1": w1h, "b1": b1h, "w2": w2h, "b2": b2h,
            "pcin": pcin,
        })
    res_e = run_bass_kernel_spmd(nc_e, in_maps, core_ids=list(range(8)), trace=trace)
    t_exp = res_e.exec_time_ns

    out_flat = np.empty((N, D), np.float32)
    for e in range(E):
        oT = res_e.results[2 * e]["outT"] + res_e.results[2 * e + 1]["outT"]
        out_flat[idx[e]] = oT[:, :len(idx[e])].T
    out = out_flat.reshape(B, S, D)
    balance_loss = np.float32(res_e.results[0]["bal"][0, 0])

    times = (t_gate, t_exp)
    return (out, balance_loss, gate_load), times


def kernel(**inputs):
    (out, balance_loss, gate_load), _ = _run(inputs, trace=False)
    return out, balance_loss, gate_load


# revision 3
# speedup vs baseline: 1.0846x; 1.0846x over previous
"""MoE layer (straight-through, gate-token routing) on 8 trn2 NeuronCores.

Strategy:
  Launch 1 (gating, data-parallel): each core takes 512 tokens (x^T shard)
    and computes logits -> softmax -> argmax mask on device. Outputs per
    core: per-token expert ids, per-(token-tile, expert) prob sums and
    counts (partition-reduced on the tensor engine via a ones matmul).
  Host: shards tokens by expert id ("all-to-all" in host numpy), 2 cores
    per expert, fixed capacity C (padded with zero columns).
  Launch 2 (experts, expert-parallel, F-split): core 2e+h holds half of
    expert e's FFN (F/2 = 1536 columns of W1, matching rows of W2) and
    processes all of expert e's tokens:
        outT_part = W2h^T @ gelu(W1h^T @ xT + b1h)   (+ b2 on h==0 core)
    Matmuls run in fp32r (full-rate fp32 mode; inputs rounded on-chip).
    The two partial outputs of a pair are summed on the host (unshard of
    the F-split) and scattered back to token positions.
  balance_loss is computed on device (launch 2) from the globally summed
  gating partials, so every output value is device-computed.
"""

import sys

if "/opt/trn_rl_repo" not in sys.path:
    sys.path.insert(0, "/opt/trn_rl_repo")

import numpy as np

import concourse.bass as bass
import concourse.mybir as mybir
import concourse.tile as tile
from concourse import bacc
from concourse.bass_utils import run_bass_kernel_spmd

F32 = mybir.dt.float32
F32R = mybir.dt.float32r
AF = mybir.ActivationFunctionType
ALU = mybir.AluOpType
AX = mybir.AxisListType

B, S, D = 8, 512, 768
E, F = 4, 3072
N = B * S                 # 4096 tokens
TPC = N // 8              # 512 tokens per core in the gating launch
DC = D // 128             # 6 contraction chunks of 128
FH = F // 2               # 1536 F-columns per core (F-split across the pair)
FT = FH // 128            # 12 f-tiles per core
C_DEFAULT = 1152          # per-expert token capacity (mean load is 1024)

_CACHE = {}


def _chunks(c):
    """Split capacity C into matmul moving-dim chunks, each in [256, 512]
    (fp32r runs full-rate only for free dim >= 256)."""
    out = []
    rem = c
    while rem >= 1024:
        out.append(512)
        rem -= 512
    if rem == 512:
        out.append(512)
    elif rem > 512:
        h1 = ((rem // 2 + 127) // 128) * 128
        out.append(h1)
        out.append(rem - h1)
    elif rem > 0:
        out.append(rem)
    spans, t0 = [], 0
    for cn in out:
        spans.append((t0, cn))
        t0 += cn
    assert t0 == c
    return spans


def _build_gate():
    nc = bacc.Bacc("TRN2", target_bir_lowering=False, debug=False, num_devices=8)
    xT = nc.dram_tensor("xT", [D, TPC], F32, kind="ExternalInput")
    gwT = nc.dram_tensor("gwT", [D, E], F32, kind="ExternalInput")
    iota = nc.dram_tensor("iota", [128, E], F32, kind="ExternalInput")
    NT = TPC // 128
    ids = nc.dram_tensor("ids", [128, NT], F32, kind="ExternalOutput")
    pc = nc.dram_tensor("pc", [2 * NT * E, 1], F32, kind="ExternalOutput")

    xr = xT.ap().rearrange("(c p) t -> c p t", p=128)
    gr = gwT.ap().rearrange("(c p) e -> p c e", p=128)

    with tile.TileContext(nc) as tc:
        with (
            tc.tile_pool(name="sb", bufs=2) as sb,
            tc.tile_pool(name="ps", bufs=4, space="PSUM") as ps,
            tc.tile_pool(name="psc", bufs=1, space="PSUM") as psc,
        ):
            xs = []
            for dcI in range(DC):
                t = sb.tile([128, TPC], F32, tag=f"x{dcI}")
                nc.sync.dma_start(t[:], xr[dcI])
                xs.append(t)
            gw = sb.tile([128, DC, E], F32, tag="gw")
            nc.sync.dma_start(gw[:], gr[:])
            io = sb.tile([128, E], F32, tag="io")
            nc.sync.dma_start(io[:], iota[:])
            ones = sb.tile([128, 1], F32, tag="ones")
            nc.vector.memset(ones[:], 1.0)

            # logits for all 4 token tiles, gathered into one [128, NT, E]
            lg = sb.tile([128, NT, E], F32, tag="lg")
            for tt in range(NT):
                pl = ps.tile([128, E], F32, tag="pl")
                for dcI in range(DC):
                    nc.tensor.matmul(
                        pl[:],
                        xs[dcI][:, tt * 128:(tt + 1) * 128],
                        gw[:, dcI, :],
                        start=(dcI == 0),
                        stop=(dcI == DC - 1),
                    )
                nc.vector.tensor_copy(lg[:, tt, :], pl[:])

            m4 = sb.tile([128, NT], F32, tag="m4")
            nc.vector.tensor_reduce(m4[:], lg[:], AX.X, ALU.max)
            mb = m4[:, :, None].broadcast_to([128, NT, E])
            # big: [probs (NT,E) | mask (NT,E)]
            big = sb.tile([128, 2 * NT * E], F32, tag="big")
            bigv = big[:].rearrange("p (k t e) -> p k t e", k=2, t=NT)
            ex = sb.tile([128, NT, E], F32, tag="ex")
            nc.vector.tensor_tensor(ex[:], lg[:], mb, ALU.subtract)
            nc.scalar.activation(ex[:], ex[:], AF.Exp)
            s4 = sb.tile([128, NT], F32, tag="s4")
            nc.vector.tensor_reduce(s4[:], ex[:], AX.X, ALU.add)
            r4 = sb.tile([128, NT], F32, tag="r4")
            nc.vector.reciprocal(r4[:], s4[:])
            rb = r4[:, :, None].broadcast_to([128, NT, E])
            nc.vector.tensor_tensor(bigv[:, 0], ex[:], rb, ALU.mult)
            nc.vector.tensor_tensor(bigv[:, 1], lg[:], mb, ALU.is_ge)
            # ids = sum_e e * mask
            iob = io[:, None, :].broadcast_to([128, NT, E])
            tmp = sb.tile([128, NT, E], F32, tag="tmp")
            nc.vector.tensor_tensor(tmp[:], bigv[:, 1], iob, ALU.mult)
            ids_sb = sb.tile([128, NT], F32, tag="ids")
            nc.vector.tensor_reduce(ids_sb[:], tmp[:], AX.X, ALU.add)
            # column sums over the 128 tokens on partitions: ones matmul
            ppc = psc.tile([2 * NT * E, 1], F32, tag="ppc")
            nc.tensor.matmul(ppc[:], big[:], ones[:], start=True, stop=True)
            pc_sb = sb.tile([2 * NT * E, 1], F32, tag="pc")
            nc.vector.tensor_copy(pc_sb[:], ppc[:])
            nc.sync.dma_start(pc[:], pc_sb[:])
            nc.sync.dma_start(ids[:], ids_sb[:])
    nc.compile()
    return nc


def _build_expert(cap):
    spans = _chunks(cap)
    NS = len(spans)
    nc = bacc.Bacc("TRN2", target_bir_lowering=False, debug=False, num_devices=8)
    xT = nc.dram_tensor("xT", [D, cap], F32, kind="ExternalInput")
    # w1 host layout: [FT, 128(p=f%128), DC, 128(d%128)] -> per-ft slab
    w1 = nc.dram_tensor("w1", [FT, 128, DC, 128], F32, kind="ExternalInput")
    b1 = nc.dram_tensor("b1", [128, FT], F32, kind="ExternalInput")
    w2 = nc.dram_tensor("w2", [FH, D], F32, kind="ExternalInput")
    b2 = nc.dram_tensor("b2", [128, DC], F32, kind="ExternalInput")
    pcin = nc.dram_tensor("pcin", [1, 2 * E], F32, kind="ExternalInput")
    outT = nc.dram_tensor("outT", [D, cap], F32, kind="ExternalOutput")
    bal = nc.dram_tensor("bal", [1, 1], F32, kind="ExternalOutput")

    xr = xT.ap().rearrange("(c p) t -> c p t", p=128)
    w2r_d = w2.ap().rearrange("(c p) d -> c p d", p=128)
    outr = outT.ap().rearrange("(c p) t -> c p t", p=128)

    with tile.TileContext(nc) as tc:
        with (
            tc.tile_pool(name="stage", bufs=3) as stage,
            tc.tile_pool(name="res", bufs=1) as res,
            tc.tile_pool(name="hp", bufs=1) as hp,
            tc.tile_pool(name="op", bufs=3) as op,
            tc.tile_pool(name="psA", bufs=3, space="PSUM") as psA,
            tc.tile_pool(name="psB", bufs=3, space="PSUM") as psB,
        ):
            b1_sb = res.tile([128, FT], F32, tag="b1")
            nc.sync.dma_start(b1_sb[:], b1.ap())
            b2_sb = res.tile([128, DC], F32, tag="b2")
            nc.sync.dma_start(b2_sb[:], b2.ap())

            # x: stream + round per (dc, span) so the first matmuls can
            # start as soon as the first span is in
            xrr = [[None] * NS for _ in range(DC)]
            for si, (t0, tn) in enumerate(spans):
                for dcI in range(DC):
                    st = stage.tile([128, tn], F32, tag="xs")
                    nc.sync.dma_start(st[:], xr[dcI][:, t0:t0 + tn])
                    rt = res.tile([128, tn], F32R, tag=f"xr{dcI}_{si}")
                    nc.vector.tensor_copy(rt[:], st[:])
                    xrr[dcI][si] = rt

            # w1: per-ft slab [128, DC, 128]
            w1t = []
            for ft in range(FT):
                st = stage.tile([128, DC, 128], F32, tag="w1s")
                nc.sync.dma_start(st[:], w1.ap()[ft])
                rt = res.tile([128, DC, 128], F32R, tag=f"w1r{ft}")
                nc.vector.tensor_copy(rt[:], st[:])
                w1t.append(rt)

            hs = []
            for ft in range(FT):
                h = hp.tile([128, cap], F32R, tag=f"h{ft}")
                hs.append(h)

            # layer 1: h = gelu(W1h^T @ xT + b1h), written as fp32r by ACT
            for ft in range(FT):
                for si, (t0, tn) in enumerate(spans):
                    ph = psA.tile([128, tn], F32, tag="ph")
                    for dcI in range(DC):
                        nc.tensor.matmul(
                            ph[:],
                            w1t[ft][:, dcI, :],
                            xrr[dcI][si][:],
                            start=(dcI == 0),
                            stop=(dcI == DC - 1),
                        )
                    nc.scalar.activation(
                        hs[ft][:, t0:t0 + tn], ph[:], AF.Gelu,
                        bias=b1_sb[:, ft:ft + 1], scale=1.0,
                    )

            # w2 loads are emitted after L1 so they don't crowd the early
            # DMA/DVE critical path; they complete during L1 compute
            w2t = []
            for fc in range(FT):
                st = stage.tile([128, D], F32, tag="w2s")
                nc.sync.dma_start(st[:], w2r_d[fc])
                rt = res.tile([128, D], F32R, tag=f"w2r{fc}")
                nc.vector.tensor_copy(rt[:], st[:])
                w2t.append(rt)

            # layer 2: outT = W2h^T @ h + b2 (d-major, per-partition bias)
            for si, (t0, tn) in enumerate(spans):
                for db in range(DC):
                    po = psB.tile([128, tn], F32, tag="po")
                    for fc in range(FT):
                        nc.tensor.matmul(
                            po[:],
                            w2t[fc][:, db * 128:(db + 1) * 128],
                            hs[fc][:, t0:t0 + tn],
                            start=(fc == 0),
                            stop=(fc == FT - 1),
                        )
                    ot = op.tile([128, tn], F32, tag="ot")
                    nc.scalar.activation(
                        ot[:], po[:], AF.Identity, bias=b2_sb[:, db:db + 1], scale=1.0,
                    )
                    nc.sync.dma_start(outr[db][:, t0:t0 + tn], ot[:])

            # balance loss from global gating partials
            pc_sb = res.tile([1, 2 * E], F32, tag="pc")
            nc.sync.dma_start(pc_sb[:], pcin.ap())
            tmp4 = res.tile([1, E], F32, tag="tmp4")
            nc.vector.tensor_tensor(tmp4[:], pc_sb[0:1, 0:E], pc_sb[0:1, E:2 * E], ALU.mult)
            bsum = res.tile([1, 1], F32, tag="bsum")
            nc.vector.tensor_reduce(bsum[:], tmp4[:], AX.X, ALU.add)
            bal_sb = res.tile([1, 1], F32, tag="bal")
            nc.scalar.mul(bal_sb[:], bsum[:], float(E) / float(N) / float(N))
            nc.sync.dma_start(bal.ap(), bal_sb[:])
    nc.compile()
    return nc


def _get_gate():
    if "gate" not in _CACHE:
        _CACHE["gate"] = _build_gate()
    return _CACHE["gate"]


def _get_expert(cap):
    key = ("exp", cap)
    if key not in _CACHE:
        _CACHE[key] = _build_expert(cap)
    return _CACHE[key]


def _run(inputs, trace=False):
    x = np.ascontiguousarray(np.asarray(inputs["x"], dtype=np.float32))
    gate_w = np.asarray(inputs["gate_w"], dtype=np.float32)
    W1 = np.asarray(inputs["W1"], dtype=np.float32)
    b1 = np.asarray(inputs["b1"], dtype=np.float32)
    W2 = np.asarray(inputs["W2"], dtype=np.float32)
    b2 = np.asarray(inputs["b2"], dtype=np.float32)

    xf = x.reshape(N, D)
    xT = np.ascontiguousarray(xf.T)                      # [768, 4096]
    gwT = np.ascontiguousarray(gate_w.T)                 # [768, 4]
    iota = np.broadcast_to(
        np.arange(E, dtype=np.float32), (128, E)
    ).copy()

    # ---- launch 1: gating ----
    nc_g = _get_gate()
    in_maps = []
    for c in range(8):
        in_maps.append({
            "xT": np.ascontiguousarray(xT[:, c * TPC:(c + 1) * TPC]),
            "gwT": gwT,
            "iota": iota,
        })
    res_g = run_bass_kernel_spmd(nc_g, in_maps, core_ids=list(range(8)), trace=trace)
    t_gate = res_g.exec_time_ns

    NT = TPC // 128
    gate = np.concatenate(
        [res_g.results[c]["ids"].T.reshape(-1) for c in range(8)]
    )
    gate = np.rint(gate).astype(np.int64)
    # pc rows: [k(probs/mask), tt, e]
    pcs = np.sum([res_g.results[c]["pc"][:, 0] for c in range(8)], axis=0)
    pcs = pcs.reshape(2, NT, E).sum(axis=1)
    probsum, counts = pcs[0], pcs[1]
    gate_load = np.rint(counts).astype(np.int32)

    # ---- host all-to-all by gate id ----
    idx = [np.flatnonzero(gate == e) for e in range(E)]
    max_load = max(len(i) for i in idx)
    cap = C_DEFAULT
    if max_load > cap:
        cap = ((max_load + 255) // 256) * 256
        if cap % 512 == 128:
            cap += 128
    nc_e = _get_expert(cap)

    pcin = np.concatenate([probsum, counts]).astype(np.float32)[None, :]
    zeros_b2 = np.zeros_like(b2[0])
    in_maps = []
    xsel = []
    for e in range(E):
        xs = np.zeros((D, cap), np.float32)
        xs[:, :len(idx[e])] = xT[:, idx[e]]
        xsel.append(xs)
    for core in range(8):
        e, h = core // 2, core % 2
        w1h = W1[e][:, h * FH:(h + 1) * FH]              # [768, 1536]
        # -> [FT, 128(d%128), DC, 128(f%128)]: lhsT needs d on partitions
        w1h = np.ascontiguousarray(
            w1h.reshape(DC, 128, FT, 128).transpose(2, 1, 0, 3)
        )
        b1h = np.ascontiguousarray(b1[e][h * FH:(h + 1) * FH].reshape(FT, 128).T)
        w2h = np.ascontiguousarray(W2[e][h * FH:(h + 1) * FH, :])
        b2h = b2[e] if h == 0 else zeros_b2
        b2h = np.ascontiguousarray(b2h.reshape(DC, 128).T)
        in_maps.append({
            "xT": xsel[e], "w1": w1h, "b1": b1h, "w2": w2h, "b2": b2h,
            "pcin": pcin,
        })
    res_e = run_bass_kernel_spmd(nc_e, in_maps, core_ids=list(range(8)), trace=trace)
    t_exp = res_e.exec_time_ns

    out_flat = np.empty((N, D), np.float32)
    for e in range(E):
        oT = res_e.results[2 * e]["outT"] + res_e.results[2 * e + 1]["outT"]
        out_flat[idx[e]] = oT[:, :len(idx[e])].T
    out = out_flat.reshape(B, S, D)
    balance_loss = np.float32(res_e.results[0]["bal"][0, 0])

    times = (t_gate, t_exp)
    return (out, balance_loss, gate_load), times


def kernel(**inputs):
    (out, balance_loss, gate_load), _ = _run(inputs, trace=False)
    return out, balance_loss, gate_load
